# revision 2
# baseline (speedup 1.0000x reference)
"""Trainium2 Bass kernel for nn_MultiHeadClassifier.

  logits[b, c] = sum_{(g,l): label_ids[g,l]==c} group_probs[b,g] *
                 (features[b] @ W[g,l] + b[g,l])

Data-parallel over batch (8 cores, 4096 rows each). Per core:
  * Host prep: pack the G*L=1024 head outputs into 8 chunks of exactly
    128 rows with NO class split across chunks (each class's rows live
    in one chunk; always feasible since a 128-row chunk holds <=128
    distinct classes).  Chunk j then owns w_j (~80) distinct classes and
    its scatter is ONE matmul: out[w_j, b] = S_j^T @ wtj_j with the
    0/1 matrix S_j [128, w_j] stationary and the 512-batch tile moving.
  * GEMM1 (PE, fp16): pg[gl, b] = W^T.T @ X^T per (chunk, b-tile),
    4 accumulating K=128 matmuls.
  * Fused (DVE): wtj = (pg + bias_j) * ptx_j via scalar_tensor_tensor,
    PSUM in, fp16 SBUF out.
  * Scatter (PE, fp16): one [w_j, 512] matmul per chunk, no PSUM-bank
    splits, no accumulation.  Emission interleaves scatter of tile t-1
    into the GEMM1 stream of tile t so PSUM drains keep pace.
  * Drain (ACT): per-chunk strip PSUM -> fp16 SBUF; strip out-DMAs on
    sync/gpsimd queues (only w_j rows each -> 5.0 MiB output/core).
Output strips are fp16 on device; host maps strip rows back to class
ids and casts to fp32.
"""
import os
import sys
import numpy as np

for _p in ("/opt/trn_rl_repo",):
    if _p not in sys.path:
        sys.path.append(_p)

import concourse.bass as bass  # noqa: E402
import concourse.tile as tile  # noqa: E402
from concourse import bacc, mybir, bass_utils  # noqa: E402
from contextlib import ExitStack  # noqa: E402

F32 = mybir.dt.float32
F16 = mybir.dt.float16

B, F, G, L, C = 32768, 512, 16, 64, 1000
NCORE = 8
BC = B // NCORE          # 4096 batch rows per core
NT = BC // 512           # 8 b-tiles of 512
KF = F // 128            # 4 feature chunks
NCH = 8                  # 8 chunks of 128 head-outputs (exact, no pad)

LAST_EXEC_NS = None


def _host_prep(W, b, label_ids):
    """Pack classes whole into 8 chunks of exactly 128 rows (best-fit
    decreasing; singletons top off each chunk to an exact fill).
    Returns packed W/bias/S plus per-chunk class lists."""
    lab = np.asarray(label_ids).reshape(-1).astype(np.int64)
    GL = lab.shape[0]
    Wflat = np.asarray(W, dtype=np.float32).reshape(GL, F)
    bflat = np.asarray(b, dtype=np.float32).reshape(GL)

    rows_of = {}
    for gl, c in enumerate(lab):
        rows_of.setdefault(int(c), []).append(gl)
    classes = sorted(rows_of.items(), key=lambda x: -len(x[1]))
    bins = [[] for _ in range(NCH)]
    caps = [128] * NCH
    for c, rows in classes:
        cand = [i for i in range(NCH) if caps[i] >= len(rows)]
        if not cand:
            raise RuntimeError("class packing failed")
        i = max(cand, key=lambda i: caps[i])
        bins[i].append(c)
        caps[i] -= len(rows)
    assert all(cp == 0 for cp in caps)

    ws = [len(bn) for bn in bins]
    s_off = np.concatenate([[0], np.cumsum(ws)]).astype(np.int64)
    SSW = int(s_off[-1])

    WT = np.zeros((F, NCH * 128), dtype=np.float16)
    biasT = np.zeros((128, NCH), dtype=np.float32)
    SS = np.zeros((128, SSW), dtype=np.float16)
    gmap = np.zeros((NCH, 128), dtype=np.int64)
    for j, bn in enumerate(bins):
        r = 0
        for m, c in enumerate(bn):
            for gl in rows_of[c]:
                WT[:, j * 128 + r] = Wflat[gl]
                biasT[r, j] = bflat[gl]
                SS[r, s_off[j] + m] = 1.0
                gmap[j, r] = gl // L
                r += 1
        assert r == 128
    return dict(WT=WT, biasT=biasT, SS=SS, SSW=SSW, gmap=gmap,
                ws=ws, s_off=s_off, bins=bins)


def _build_program(SSW, ws, s_off):
    nc = bacc.Bacc("TRN2", target_bir_lowering=False, debug=False,
                   num_devices=NCORE)
    # xk: k-interleaved X^T packed by t-pair — row blocks hold
    # (t1,t2), (t3,t4), (t5,t6), (t7, pad). t0 rides inside wg.
    xk_d = nc.dram_tensor("xk", [(NT // 2) * 128, 2 * KF * 512], F16,
                          kind="ExternalInput").ap()
    # ptx: expanded group probs + 8 bias columns. Layout: chunks 0-3
    # (2048), all 8 bias cols (8), chunks 4-7 (2048) — so the first-half
    # DMA already carries every bias column.
    PW = NCH * 512 + 8
    ptx_d = nc.dram_tensor("ptx", [NT * 128, PW], F16,
                           kind="ExternalInput").ap()
    # wg: the startup gate — X^T t0 (k-interleaved, 2048 cols) plus
    # chunk-0's W blocks (k0..k3, 512 cols). ONE small first DMA clears
    # the whole first-GEMM gate.
    wg_d = nc.dram_tensor("wg", [128, KF * 512 + KF * 128], F16,
                          kind="ExternalInput").ap()
    # wrest: W chunks 1..7, j-major: col (j-1)*512 + k*128.
    wrest_d = nc.dram_tensor("wrest", [128, (NCH - 1) * KF * 128], F16,
                             kind="ExternalInput").ap()
    s_d = nc.dram_tensor("s", [128, SSW], F16, kind="ExternalInput").ap()
    # out strips: block (t, j) at cols (t*8+j)*512, rows 0..w_j hold
    # logits[t*512+b, cls_j[m]] transposed (class-major); host unpacks.
    out_d = nc.dram_tensor("logits", [128, NT * NCH * 512], F16,
                           kind="ExternalOutput").ap()

    with tile.TileContext(nc) as tc, ExitStack() as ctx:
        const = ctx.enter_context(tc.tile_pool(name="const", bufs=1))
        psG = ctx.enter_context(tc.tile_pool(name="psG", bufs=3, space="PSUM"))
        psL = ctx.enter_context(tc.tile_pool(name="psL", bufs=3, space="PSUM"))
        sbW = ctx.enter_context(tc.tile_pool(name="sbW", bufs=18))
        sbO = ctx.enter_context(tc.tile_pool(name="sbO", bufs=6))

        # startup gate first on the sync queue, then the rest
        wg = const.tile([128, KF * 512 + KF * 128], F16, name="wg", tag="wg")
        nc.sync.dma_start(wg[:], wg_d[:])
        wrest = const.tile([128, (NCH - 1) * KF * 128], F16, name="wrest",
                           tag="wrest")
        nc.sync.dma_start(wrest[:], wrest_d[:])
        ss = const.tile([128, SSW], F16, name="ss", tag="ss")
        nc.sync.dma_start(ss[:], s_d[:])
        xtile = [None] * NT     # (tile, col_base) per t
        xtile[0] = (wg, 0)
        xp0 = const.tile([128, 4096], F16, name="xp0", tag="xp0")
        nc.sync.dma_start(xp0[:], xk_d[0:128, :])
        xtile[1] = (xp0, 0)
        xtile[2] = (xp0, 2048)
        for bi in range(1, 3):
            t_ = const.tile([128, 4096], F16, name=f"xp{bi}", tag=f"xp{bi}")
            nc.sync.dma_start(t_[:], xk_d[bi * 128:(bi + 1) * 128, :])
            xtile[2 * bi + 1] = (t_, 0)
            xtile[2 * bi + 2] = (t_, 2048)
        x7 = const.tile([128, 2048], F16, name="x7", tag="x7")
        nc.sync.dma_start(x7[:], xk_d[3 * 128:4 * 128, 0:2048])
        xtile[7] = (x7, 0)
        # expanded group probs (+bias cols). t0 split in two on scalar
        # so the first stt gates on only 514KB; t1-7 whole on gpsimd.
        HB = 2048 + 8            # start of the chunk 4-7 columns
        ptx0a = const.tile([128, 2056], F16, name="ptx0a", tag="ptx0a")
        nc.scalar.dma_start(ptx0a[:], ptx_d[0:128, 0:2056])
        ptx0b = const.tile([128, 2048], F16, name="ptx0b", tag="ptx0b")
        nc.scalar.dma_start(ptx0b[:], ptx_d[0:128, 2056:PW])
        ptxs = [None] * NT
        for t in range(1, NT):
            t_ = const.tile([128, PW], F16, name=f"ptx{t}", tag=f"ptx{t}")
            nc.gpsimd.dma_start(t_[:], ptx_d[t * 128:(t + 1) * 128, :])
            ptxs[t] = t_

        def w_ap(j, k):
            if j == 0:
                return wg[:, KF * 512 + k * 128:KF * 512 + (k + 1) * 128]
            return wrest[:, (j - 1) * KF * 128 + k * 128:
                         (j - 1) * KF * 128 + (k + 1) * 128]

        def ptx_ap(t, j):
            if t == 0:
                if j < 4:
                    return ptx0a[:, j * 512:(j + 1) * 512]
                return ptx0b[:, (j - 4) * 512:(j - 3) * 512]
            pcol = j * 512 if j < 4 else HB + (j - 4) * 512
            return ptxs[t][:, pcol:pcol + 512]

        def bias_ap(t, j):
            if t == 0:
                return ptx0a[:, 2048 + j:2049 + j]
            return ptxs[t][:, 2048 + j:2049 + j]

        wtj_prev = [None] * NCH
        wtj_cur = [None] * NCH
        for t in range(NT + 1):
            for j in range(NCH):
                if t < NT:
                    xt_, xb = xtile[t]
                    pg = psG.tile([128, 512], F32, name="pg", tag="pg")
                    for k in range(KF):
                        nc.tensor.matmul(pg[:], w_ap(j, k),
                                         xt_[:, xb + k * 512:
                                             xb + (k + 1) * 512],
                                         start=(k == 0), stop=(k == KF - 1))
                    wtj = sbW.tile([128, 512], F16, name="wtj", tag="wtj")
                    nc.vector.scalar_tensor_tensor(
                        wtj[:], pg[:], bias_ap(t, j), ptx_ap(t, j),
                        op0=mybir.AluOpType.add, op1=mybir.AluOpType.mult)
                    wtj_cur[j] = wtj
                if t >= 1:
                    w_j = ws[j]
                    so = int(s_off[j])
                    pl = psL.tile([128, 512], F32, name="pl", tag="pl")
                    nc.tensor.matmul(pl[0:w_j, :], ss[:, so:so + w_j],
                                     wtj_prev[j][:], start=True, stop=True)
                    obs = sbO.tile([128, 512], F16, name="obs", tag="obs")
                    # tail iteration: split drains across ACT and DVE
                    deng = nc.vector if (t == NT and j % 2 == 1) else nc.scalar
                    if deng is nc.scalar:
                        deng.activation(obs[0:w_j, :], pl[0:w_j, :],
                                        mybir.ActivationFunctionType.Copy,
                                        bias=0.0, scale=1.0)
                    else:
                        deng.tensor_copy(obs[0:w_j, :], pl[0:w_j, :])
                    oeng = nc.sync if j % 2 == 0 else nc.gpsimd
                    col = ((t - 1) * NCH + j) * 512
                    oeng.dma_start(out_d[0:w_j, col:col + 512],
                                   obs[0:w_j, :])
            wtj_prev, wtj_cur = wtj_cur, wtj_prev
    nc.finalize()
    return nc


def kernel(features, group_probs, W, b, label_ids):
    global LAST_EXEC_NS
    features = np.asarray(features, dtype=np.float32)
    group_probs = np.asarray(group_probs, dtype=np.float32)
    prep = _host_prep(W, b, label_ids)
    ws, s_off, bins = prep["ws"], prep["s_off"], prep["bins"]
    nc = _build_program(prep["SSW"], ws, s_off)

    XT = features.T.astype(np.float16)                        # [F, B]
    PT = group_probs.T.astype(np.float16)                     # [G, B]
    gmap = prep["gmap"]
    WTf = prep["WT"]                                          # [F, 1024]
    # wg/wrest: j-major W blocks, col (j)*512 + k*128 = WTf[k*128.., j*128..]
    wj = np.empty((128, NCH * KF * 128), dtype=np.float16)
    for j in range(NCH):
        for k in range(KF):
            wj[:, j * 512 + k * 128:j * 512 + (k + 1) * 128] = \
                WTf[k * 128:(k + 1) * 128, j * 128:(j + 1) * 128]
    bias16 = prep["biasT"].astype(np.float16)                 # [128, NCH]
    in_maps = []
    for c in range(NCORE):
        # k-interleaved X^T: xflat[t, p, k*512+cc] = XT[k*128+p, t*512+cc]
        xc = XT[:, c * BC:(c + 1) * BC].reshape(KF, 128, NT, 512)
        xflat = xc.transpose(2, 1, 0, 3).reshape(NT, 128, KF * 512)
        # wg = X t0 + W chunk 0; wrest = W chunks 1..7
        wgc = np.concatenate([xflat[0], wj[:, 0:512]], axis=1)
        wrestc = np.ascontiguousarray(wj[:, 512:])
        # xk row-blocks: (t1,t2), (t3,t4), (t5,t6), (t7, zero-pad)
        xk = np.zeros(((NT // 2) * 128, 2 * KF * 512), dtype=np.float16)
        for bi in range(3):
            xk[bi * 128:(bi + 1) * 128, :2048] = xflat[2 * bi + 1]
            xk[bi * 128:(bi + 1) * 128, 2048:] = xflat[2 * bi + 2]
        xk[3 * 128:4 * 128, :2048] = xflat[7]
        xk = np.ascontiguousarray(xk)
        ptc = PT[:, c * BC:(c + 1) * BC].reshape(G, NT, 512)  # [16, 8, 512]
        ptx = np.empty((NT, 128, NCH * 512 + 8), dtype=np.float16)
        for j in range(NCH):
            pcol = j * 512 if j < 4 else 2056 + (j - 4) * 512
            ptx[:, :, pcol:pcol + 512] = ptc[gmap[j]].transpose(1, 0, 2)
        ptx[:, :, 2048:2056] = bias16[None, :, :]
        in_maps.append({
            "xk": xk,
            "ptx": np.ascontiguousarray(ptx.reshape(NT * 128, -1)),
            "wg": np.ascontiguousarray(wgc),
            "wrest": wrestc,
            "s": prep["SS"],
        })

    trace = bool(os.environ.get("BASS_TRACE"))
    if trace:
        bass_utils.upload_artifacts = lambda d: "local://skipped"
    try:
        res = bass_utils.run_bass_kernel_spmd(nc, in_maps,
                                              core_ids=list(range(NCORE)))
    except Exception:
        # transient NRT device errors have been observed; one retry
        res = bass_utils.run_bass_kernel_spmd(nc, in_maps,
                                              core_ids=list(range(NCORE)))
    if trace:
        LAST_EXEC_NS = res.exec_time_ns
        if res.exec_time_ns is not None:
            print(f"HW exec time: {res.exec_time_ns} ns")

    out = np.zeros((B, C), dtype=np.float32)
    for c in range(NCORE):
        o2 = res.results[c]["logits"]                          # [128, 32768]
        r0 = c * BC
        for t in range(NT):
            for j in range(NCH):
                col = (t * NCH + j) * 512
                strip = o2[0:ws[j], col:col + 512]             # [w_j, 512]
                out[r0 + t * 512:r0 + (t + 1) * 512, bins[j]] = \
                    strip.T.astype(np.float32)
    return out


# revision 4
# speedup vs baseline: 1.0824x; 1.0824x over previous
"""Trainium2 Bass kernel for nn_MultiHeadClassifier.

  logits[b, c] = sum_{(g,l): label_ids[g,l]==c} group_probs[b,g] *
                 (features[b] @ W[g,l] + b[g,l])

Data-parallel over batch (8 cores, 4096 rows each). Per core:
  * Host prep: pack the G*L=1024 head outputs into 8 chunks of exactly
    128 rows with NO class split across chunks (each class's rows live
    in one chunk; always feasible since a 128-row chunk holds <=128
    distinct classes).  Chunk j owns w_j (~80) distinct classes and its
    scatter is ONE matmul: out[128, b] = S_j^T @ wtj_j with the 0/1
    matrix S_j [128, 128] (zero-padded cols) stationary and the
    512-batch tile moving — rows w_j..128 of the result are zeros.
  * GEMM1 (PE, fp16): pg[gl, b] = W^T.T @ X^T per (chunk, b-tile),
    4 accumulating K=128 matmuls.  W streams in per-chunk 128KB DMAs so
    the PE never waits on weights; PE warm-up matmuls during the gate
    DMA keep the p-state ramp off the critical path.
  * Fused (DVE): wtj = (pg + bias_j) * ptx_j via scalar_tensor_tensor.
  * Scatter (PE, fp16): one [128, 512] matmul per chunk, interleaved
    into the next tile's GEMM1 stream so PSUM drains keep pace.
  * Drain (ACT): per-chunk strip PSUM -> fp16 into a per-tile [128,
    4096] SBUF tile; one 1MB out-DMA per tile, queue alternating by
    tile parity.  ptx tiles are issued inside the loop (staggered) so
    startup HBM bandwidth goes to the X/W critical path.
Output strips are fp16; host maps strip rows back to class ids.
"""
import os
import sys
import numpy as np

for _p in ("/opt/trn_rl_repo",):
    if _p not in sys.path:
        sys.path.append(_p)

import concourse.bass as bass  # noqa: E402
import concourse.tile as tile  # noqa: E402
from concourse import bacc, mybir, bass_utils  # noqa: E402
from contextlib import ExitStack  # noqa: E402

F32 = mybir.dt.float32
F16 = mybir.dt.float16

B, F, G, L, C = 32768, 512, 16, 64, 1000
NCORE = 8
BC = B // NCORE          # 4096 batch rows per core
NT = BC // 512           # 8 b-tiles of 512
KF = F // 128            # 4 feature chunks
NCH = 8                  # 8 chunks of 128 head-outputs (exact, no pad)
NWARM = 36               # PE p-state warm-up matmuls during the gate DMA

LAST_EXEC_NS = None


def _host_prep(W, b, label_ids):
    """Pack classes whole into 8 chunks of exactly 128 rows (best-fit
    decreasing; singletons top off each chunk to an exact fill).
    Returns packed W/bias/S plus per-chunk class lists."""
    lab = np.asarray(label_ids).reshape(-1).astype(np.int64)
    GL = lab.shape[0]
    Wflat = np.asarray(W, dtype=np.float32).reshape(GL, F)
    bflat = np.asarray(b, dtype=np.float32).reshape(GL)

    rows_of = {}
    for gl, c in enumerate(lab):
        rows_of.setdefault(int(c), []).append(gl)
    classes = sorted(rows_of.items(), key=lambda x: -len(x[1]))
    bins = [[] for _ in range(NCH)]
    caps = [128] * NCH
    for c, rows in classes:
        cand = [i for i in range(NCH) if caps[i] >= len(rows)]
        if not cand:
            raise RuntimeError("class packing failed")
        i = max(cand, key=lambda i: caps[i])
        bins[i].append(c)
        caps[i] -= len(rows)
    assert all(cp == 0 for cp in caps)

    ws = [len(bn) for bn in bins]
    WT = np.zeros((F, NCH * 128), dtype=np.float16)
    biasT = np.zeros((128, NCH), dtype=np.float32)
    SS = np.zeros((128, NCH * 128), dtype=np.float16)  # per-chunk padded
    gmap = np.zeros((NCH, 128), dtype=np.int64)
    for j, bn in enumerate(bins):
        r = 0
        for m, c in enumerate(bn):
            for gl in rows_of[c]:
                WT[:, j * 128 + r] = Wflat[gl]
                biasT[r, j] = bflat[gl]
                SS[r, j * 128 + m] = 1.0
                gmap[j, r] = gl // L
                r += 1
        assert r == 128
    return dict(WT=WT, biasT=biasT, SS=SS, gmap=gmap, ws=ws, bins=bins)


def _build_program():
    nc = bacc.Bacc("TRN2", target_bir_lowering=False, debug=False,
                   num_devices=NCORE)
    # xk: k-interleaved X^T packed by t-pair — row blocks hold
    # (t1,t2), (t3,t4), (t5,t6), (t7, pad). t0 rides inside wg.
    xk_d = nc.dram_tensor("xk", [(NT // 2) * 128, 2 * KF * 512], F16,
                          kind="ExternalInput").ap()
    # ptx: expanded group probs + 8 bias columns. Layout: chunks 0-3
    # (2048), all 8 bias cols (8), chunks 4-7 (2048) — so the first-half
    # DMA already carries every bias column.
    PW = NCH * 512 + 8
    ptx_d = nc.dram_tensor("ptx", [NT * 128, PW], F16,
                           kind="ExternalInput").ap()
    # wg: the startup gate — X^T t0 (k-interleaved, 2048 cols) plus
    # chunk-0's W blocks (k0..k3, 512 cols).
    wg_d = nc.dram_tensor("wg", [128, KF * 512 + KF * 128], F16,
                          kind="ExternalInput").ap()
    # wrest: W chunks 1..7, j-major: col (j-1)*512 + k*128; DMA'd in
    # per-chunk 128KB pieces so chunk j+1's weights land while chunk j
    # computes.
    wrest_d = nc.dram_tensor("wrest", [128, (NCH - 1) * KF * 128], F16,
                             kind="ExternalInput").ap()
    s_d = nc.dram_tensor("s", [128, NCH * 128], F16,
                         kind="ExternalInput").ap()
    # out: block (t, j) at cols (t*8+j)*512; rows 0..w_j are the w_j
    # class strips of chunk j (transposed, class-major); rest zeros.
    out_d = nc.dram_tensor("logits", [128, NT * NCH * 512], F16,
                           kind="ExternalOutput").ap()

    with tile.TileContext(nc) as tc, ExitStack() as ctx:
        const = ctx.enter_context(tc.tile_pool(name="const", bufs=1))
        psG = ctx.enter_context(tc.tile_pool(name="psG", bufs=3, space="PSUM"))
        psL = ctx.enter_context(tc.tile_pool(name="psL", bufs=3, space="PSUM"))
        psW = ctx.enter_context(tc.tile_pool(name="psW", bufs=1, space="PSUM"))
        sbW = ctx.enter_context(tc.tile_pool(name="sbW", bufs=18))
        sbO = ctx.enter_context(tc.tile_pool(name="sbO", bufs=3))

        # PE warm-up: ramp the tensor-engine p-state while the gate DMA
        # streams.  Source is a memset tile; results are never read.
        warm = const.tile([128, 256], F16, name="warm", tag="warm")
        nc.gpsimd.memset(warm[:], 0.0)
        wps = psW.tile([128, 128], F32, name="wps", tag="wps")
        for _ in range(NWARM):
            nc.tensor.matmul(wps[:], warm[:, 0:128], warm[:, 128:256],
                             start=True, stop=True)

        # gate first on the sync queue, then per-chunk W, ss, X pairs
        wg = const.tile([128, KF * 512 + KF * 128], F16, name="wg", tag="wg")
        nc.sync.dma_start(wg[:], wg_d[:])
        wr = [None] * NCH
        for j in range(1, NCH):
            t_ = const.tile([128, KF * 128], F16, name=f"wr{j}", tag=f"wr{j}")
            nc.sync.dma_start(t_[:], wrest_d[:, (j - 1) * 512:j * 512])
            wr[j] = t_
        ss = const.tile([128, NCH * 128], F16, name="ss", tag="ss")
        nc.sync.dma_start(ss[:], s_d[:])
        xtile = [None] * NT     # (tile, col_base) per t
        xtile[0] = (wg, 0)
        xp0 = const.tile([128, 4096], F16, name="xp0", tag="xp0")
        nc.sync.dma_start(xp0[:], xk_d[0:128, :])
        xtile[1] = (xp0, 0)
        xtile[2] = (xp0, 2048)
        for bi in range(1, 3):
            t_ = const.tile([128, 4096], F16, name=f"xp{bi}", tag=f"xp{bi}")
            nc.sync.dma_start(t_[:], xk_d[bi * 128:(bi + 1) * 128, :])
            xtile[2 * bi + 1] = (t_, 0)
            xtile[2 * bi + 2] = (t_, 2048)
        x7 = const.tile([128, 2048], F16, name="x7", tag="x7")
        nc.sync.dma_start(x7[:], xk_d[3 * 128:4 * 128, 0:2048])
        xtile[7] = (x7, 0)
        # group probs: t0 split in two on scalar (first stt gates on
        # 514KB); ptx1/2 upfront on gpsimd; the rest staggered in-loop.
        HB = 2048 + 8            # start of the chunk 4-7 columns
        ptx0a = const.tile([128, 2056], F16, name="ptx0a", tag="ptx0a")
        nc.scalar.dma_start(ptx0a[:], ptx_d[0:128, 0:2056])
        ptx0b = const.tile([128, 2048], F16, name="ptx0b", tag="ptx0b")
        nc.scalar.dma_start(ptx0b[:], ptx_d[0:128, 2056:PW])
        ptxs = [None] * NT
        for t in range(1, NT):
            ptxs[t] = const.tile([128, PW], F16, name=f"ptx{t}",
                                 tag=f"ptx{t}")
        for t in (1, 2):
            nc.gpsimd.dma_start(ptxs[t][:], ptx_d[t * 128:(t + 1) * 128, :])

        def w_ap(j, k):
            if j == 0:
                return wg[:, KF * 512 + k * 128:KF * 512 + (k + 1) * 128]
            return wr[j][:, k * 128:(k + 1) * 128]

        def ptx_ap(t, j):
            if t == 0:
                if j < 4:
                    return ptx0a[:, j * 512:(j + 1) * 512]
                return ptx0b[:, (j - 4) * 512:(j - 3) * 512]
            pcol = j * 512 if j < 4 else HB + (j - 4) * 512
            return ptxs[t][:, pcol:pcol + 512]

        def bias_ap(t, j):
            if t == 0:
                return ptx0a[:, 2048 + j:2049 + j]
            return ptxs[t][:, 2048 + j:2049 + j]

        wtj_prev = [None] * NCH
        wtj_cur = [None] * NCH
        obt = None
        for t in range(NT + 1):
            # staggered ptx issue: tile t+2's probs (gpsimd early,
            # sync once its input queue has drained)
            pt = t + 2
            if 3 <= pt < NT:
                eng = nc.gpsimd if pt <= 4 else nc.sync
                eng.dma_start(ptxs[pt][:], ptx_d[pt * 128:(pt + 1) * 128, :])
            if t >= 1:
                obt = sbO.tile([128, NCH * 512], F16, name="obt", tag="obt")
            for j in range(NCH):
                if t < NT:
                    xt_, xb = xtile[t]
                    pg = psG.tile([128, 512], F32, name="pg", tag="pg")
                    for k in range(KF):
                        nc.tensor.matmul(pg[:], w_ap(j, k),
                                         xt_[:, xb + k * 512:
                                             xb + (k + 1) * 512],
                                         start=(k == 0), stop=(k == KF - 1))
                    wtj = sbW.tile([128, 512], F16, name="wtj", tag="wtj")
                    nc.vector.scalar_tensor_tensor(
                        wtj[:], pg[:], bias_ap(t, j), ptx_ap(t, j),
                        op0=mybir.AluOpType.add, op1=mybir.AluOpType.mult)
                    wtj_cur[j] = wtj
                if t >= 1:
                    pl = psL.tile([128, 512], F32, name="pl", tag="pl")
                    nc.tensor.matmul(pl[:], ss[:, j * 128:(j + 1) * 128],
                                     wtj_prev[j][:], start=True, stop=True)
                    # tail iteration: split drains across ACT and DVE
                    if t == NT and j % 2 == 1:
                        nc.vector.tensor_copy(obt[:, j * 512:(j + 1) * 512],
                                              pl[:])
                    else:
                        nc.scalar.activation(obt[:, j * 512:(j + 1) * 512],
                                             pl[:],
                                             mybir.ActivationFunctionType.Copy,
                                             bias=0.0, scale=1.0)
            if t >= 1:
                col = (t - 1) * NCH * 512
                if t == NT:
                    # final tile: two half DMAs on both queues
                    nc.scalar.dma_start(out_d[:, col:col + 2048],
                                        obt[:, 0:2048])
                    nc.gpsimd.dma_start(out_d[:, col + 2048:col + 4096],
                                        obt[:, 2048:4096])
                else:
                    oeng = nc.scalar if t % 2 == 1 else nc.gpsimd
                    oeng.dma_start(out_d[:, col:col + NCH * 512], obt[:])
            if t < NT:
                wtj_prev, wtj_cur = wtj_cur, wtj_prev
    nc.finalize()
    return nc


def kernel(features, group_probs, W, b, label_ids):
    global LAST_EXEC_NS
    features = np.asarray(features, dtype=np.float32)
    group_probs = np.asarray(group_probs, dtype=np.float32)
    prep = _host_prep(W, b, label_ids)
    ws, bins = prep["ws"], prep["bins"]
    nc = _build_program()

    XT = features.T.astype(np.float16)                        # [F, B]
    PT = group_probs.T.astype(np.float16)                     # [G, B]
    gmap = prep["gmap"]
    WTf = prep["WT"]                                          # [F, 1024]
    # j-major W blocks, col j*512 + k*128 = WTf[k*128.., j*128..]
    wj = np.empty((128, NCH * KF * 128), dtype=np.float16)
    for j in range(NCH):
        for k in range(KF):
            wj[:, j * 512 + k * 128:j * 512 + (k + 1) * 128] = \
                WTf[k * 128:(k + 1) * 128, j * 128:(j + 1) * 128]
    bias16 = prep["biasT"].astype(np.float16)                 # [128, NCH]
    in_maps = []
    for c in range(NCORE):
        # k-interleaved X^T: xflat[t, p, k*512+cc] = XT[k*128+p, t*512+cc]
        xc = XT[:, c * BC:(c + 1) * BC].reshape(KF, 128, NT, 512)
        xflat = xc.transpose(2, 1, 0, 3).reshape(NT, 128, KF * 512)
        # wg = X t0 + W chunk 0; wrest = W chunks 1..7
        wgc = np.concatenate([xflat[0], wj[:, 0:512]], axis=1)
        wrestc = np.ascontiguousarray(wj[:, 512:])
        # xk row-blocks: (t1,t2), (t3,t4), (t5,t6), (t7, zero-pad)
        xk = np.zeros(((NT // 2) * 128, 2 * KF * 512), dtype=np.float16)
        for bi in range(3):
            xk[bi * 128:(bi + 1) * 128, :2048] = xflat[2 * bi + 1]
            xk[bi * 128:(bi + 1) * 128, 2048:] = xflat[2 * bi + 2]
        xk[3 * 128:4 * 128, :2048] = xflat[7]
        xk = np.ascontiguousarray(xk)
        ptc = PT[:, c * BC:(c + 1) * BC].reshape(G, NT, 512)  # [16, 8, 512]
        ptx = np.empty((NT, 128, NCH * 512 + 8), dtype=np.float16)
        for j in range(NCH):
            pcol = j * 512 if j < 4 else 2056 + (j - 4) * 512
            ptx[:, :, pcol:pcol + 512] = ptc[gmap[j]].transpose(1, 0, 2)
        ptx[:, :, 2048:2056] = bias16[None, :, :]
        in_maps.append({
            "xk": xk,
            "ptx": np.ascontiguousarray(ptx.reshape(NT * 128, -1)),
            "wg": np.ascontiguousarray(wgc),
            "wrest": wrestc,
            "s": prep["SS"],
        })

    trace = bool(os.environ.get("BASS_TRACE"))
    if trace:
        bass_utils.upload_artifacts = lambda d: "local://skipped"
    try:
        res = bass_utils.run_bass_kernel_spmd(nc, in_maps,
                                              core_ids=list(range(NCORE)))
    except Exception:
        # transient NRT device errors have been observed; one retry
        res = bass_utils.run_bass_kernel_spmd(nc, in_maps,
                                              core_ids=list(range(NCORE)))
    if trace:
        LAST_EXEC_NS = res.exec_time_ns
        if res.exec_time_ns is not None:
            print(f"HW exec time: {res.exec_time_ns} ns")

    out = np.zeros((B, C), dtype=np.float32)
    for c in range(NCORE):
        o2 = res.results[c]["logits"]                          # [128, 32768]
        r0 = c * BC
        for t in range(NT):
            for j in range(NCH):
                col = (t * NCH + j) * 512
                strip = o2[0:ws[j], col:col + 512]             # [w_j, 512]
                out[r0 + t * 512:r0 + (t + 1) * 512, bins[j]] = \
                    strip.T.astype(np.float32)
    return out


# revision 6
# speedup vs baseline: 1.2238x; 1.1305x over previous
"""Trainium2 Bass kernel for nn_MultiHeadClassifier.

  logits[b, c] = sum_{(g,l): label_ids[g,l]==c} group_probs[b,g] *
                 (features[b] @ W[g,l] + b[g,l])

Data-parallel over batch (8 cores, 4096 rows each). Per core:
  * Host prep: pack the G*L=1024 head outputs into 8 chunks of exactly
    128 rows with (a) no class split across chunks and (b) a BALANCED
    group profile — every chunk holds exactly 8 rows of each of the 16
    groups, at the same row positions (row r belongs to group r//8).
    (a) makes each chunk's scatter ONE matmul (S_j [128,128] 0/1,
    zero-padded, stationary; batch moving); (b) lets all 8 chunks share
    one [128, 512] probs tile per b-tile (8x less HBM than per-chunk
    expansion).
  * GEMM1 (PE, fp16): pg[gl, b] = W^T.T @ X^T per (chunk, b-tile),
    4 accumulating K=128 matmuls.  The gate DMA rides the
    earliest-starting (gpsimd software-DGE) queue and PE warm-up
    matmuls keep the p-state/power ramp off the critical path; W
    streams in per-chunk 128KB DMAs just ahead of consumption.
  * Fused (DVE): wtj = (pg + bias_j) * ptx via scalar_tensor_tensor.
  * Scatter (PE, fp16): one [128, 512] matmul per chunk, interleaved
    into the next tile's GEMM1 stream so PSUM drains keep pace.
  * Drain (ACT): per-chunk strip PSUM -> fp16 into a per-tile [128,
    4096] SBUF tile; one 1MB out-DMA per tile alternating sync/scalar
    queues (quarter DMAs on the final tile to shorten the tail).
Output strips are fp16; host maps strip rows back to class ids.
"""
import os
import sys
import numpy as np

for _p in ("/opt/trn_rl_repo",):
    if _p not in sys.path:
        sys.path.append(_p)

import concourse.bass as bass  # noqa: E402
import concourse.tile as tile  # noqa: E402
from concourse import bacc, mybir, bass_utils  # noqa: E402
from contextlib import ExitStack  # noqa: E402

F32 = mybir.dt.float32
F16 = mybir.dt.float16

B, F, G, L, C = 32768, 512, 16, 64, 1000
NCORE = 8
BC = B // NCORE          # 4096 batch rows per core
NT = BC // 512           # 8 b-tiles of 512
KF = F // 128            # 4 feature chunks
NCH = 8                  # 8 chunks of 128 head-outputs (exact, no pad)
GPC = 128 // G           # rows per group per chunk (8)
NWARM = 36               # PE p-state warm-up matmuls during the gate DMA
PW = 512 + NCH           # shared probs tile width (+8 bias cols)

LAST_EXEC_NS = None


def _host_prep(W, b, label_ids):
    """Pack classes whole into 8 chunks of exactly 128 rows with every
    chunk holding exactly GPC rows of each group (row r <-> group r//GPC
    for ALL chunks).  Returns packed W/bias/S plus per-chunk class
    lists."""
    lab = np.asarray(label_ids).reshape(-1).astype(np.int64)
    GL = lab.shape[0]
    Wflat = np.asarray(W, dtype=np.float32).reshape(GL, F)
    bflat = np.asarray(b, dtype=np.float32).reshape(GL)

    rows_of = {}
    for gl, c in enumerate(lab):
        rows_of.setdefault(int(c), []).append(gl)
    # class -> per-group row-count vector
    cvec = {}
    for c, rows in rows_of.items():
        v = np.zeros(G, dtype=np.int64)
        for gl in rows:
            v[gl // L] += 1
        cvec[c] = v
    target = np.full(G, GPC, dtype=np.int64)
    loads = [np.zeros(G, dtype=np.int64) for _ in range(NCH)]
    bins = [[] for _ in range(NCH)]
    order = sorted(rows_of, key=lambda c: (-len(rows_of[c]),
                                           tuple(-cvec[c])))
    for c in order:
        v = cvec[c]
        best, bestslack = None, -1
        for i in range(NCH):
            if np.all(loads[i] + v <= target):
                slack = int((target - loads[i]).sum())
                if slack > bestslack:
                    best, bestslack = i, slack
        if best is None:
            raise RuntimeError("balanced class packing failed")
        loads[best] += v
        bins[best].append(c)
    assert all(np.all(ld == target) for ld in loads)

    ws = [len(bn) for bn in bins]
    WT = np.zeros((F, NCH * 128), dtype=np.float16)
    biasT = np.zeros((128, NCH), dtype=np.float32)
    SS = np.zeros((128, NCH * 128), dtype=np.float16)  # per-chunk padded
    for j, bn in enumerate(bins):
        slot = [g * GPC for g in range(G)]   # next free row per group
        for m, c in enumerate(bn):
            for gl in rows_of[c]:
                g = gl // L
                r = slot[g]
                slot[g] += 1
                WT[:, j * 128 + r] = Wflat[gl]
                biasT[r, j] = bflat[gl]
                SS[r, j * 128 + m] = 1.0
        assert slot == [(g + 1) * GPC for g in range(G)]
    return dict(WT=WT, biasT=biasT, SS=SS, ws=ws, bins=bins)


def _build_program():
    nc = bacc.Bacc("TRN2", target_bir_lowering=False, debug=False,
                   num_devices=NCORE)
    # xk: k-interleaved X^T packed by t-pair — row blocks hold
    # (t1,t2), (t3,t4), (t5,t6), (t7, pad). t0 rides inside wg.
    xk_d = nc.dram_tensor("xk", [(NT // 2) * 128, 2 * KF * 512], F16,
                          kind="ExternalInput").ap()
    # ptx: shared probs per b-tile (row r -> group r//GPC) + 8 bias cols
    ptx_d = nc.dram_tensor("ptx", [NT * 128, PW], F16,
                           kind="ExternalInput").ap()
    # wg: the startup gate — X^T t0 (k-interleaved, 2048 cols) plus
    # chunk-0's W blocks (k0..k3, 512 cols).
    wg_d = nc.dram_tensor("wg", [128, KF * 512 + KF * 128], F16,
                          kind="ExternalInput").ap()
    # wrest: W chunks 1..7, j-major: col (j-1)*512 + k*128; DMA'd in
    # per-chunk 128KB pieces so chunk j+1's weights land while chunk j
    # computes.
    wrest_d = nc.dram_tensor("wrest", [128, (NCH - 1) * KF * 128], F16,
                             kind="ExternalInput").ap()
    s_d = nc.dram_tensor("s", [128, NCH * 128], F16,
                         kind="ExternalInput").ap()
    # out: block (t, j) at cols (t*8+j)*512; rows 0..w_j are the w_j
    # class strips of chunk j (transposed, class-major).
    out_d = nc.dram_tensor("logits", [128, NT * NCH * 512], F16,
                           kind="ExternalOutput").ap()

    with tile.TileContext(nc) as tc, ExitStack() as ctx:
        const = ctx.enter_context(tc.tile_pool(name="const", bufs=1))
        psG = ctx.enter_context(tc.tile_pool(name="psG", bufs=5, space="PSUM"))
        psL = ctx.enter_context(tc.tile_pool(name="psL", bufs=3, space="PSUM"))
        sbW = ctx.enter_context(tc.tile_pool(name="sbW", bufs=18))
        sbO = ctx.enter_context(tc.tile_pool(name="sbO", bufs=4))

        # gate chain on the gpsimd (software-DGE) queue — it starts
        # earliest.  wg gates the first real matmul; per-chunk W pieces
        # stream just ahead of the j-loop.
        wg = const.tile([128, KF * 512 + KF * 128], F16, name="wg", tag="wg")
        nc.gpsimd.dma_start(wg[:], wg_d[:])
        wr = [None] * NCH
        for j in range(1, NCH):
            t_ = const.tile([128, KF * 128], F16, name=f"wr{j}", tag=f"wr{j}")
            nc.gpsimd.dma_start(t_[:], wrest_d[:, (j - 1) * 512:j * 512])
            wr[j] = t_

        # PE warm-up: ramp the tensor-engine p-state while the gate DMA
        # streams.  Source is a memset tile; results are never read.
        warm = const.tile([128, 384], F16, name="warm", tag="warm")
        nc.vector.memset(warm[:], 0.0)
        for i in range(NWARM):
            wps = psL.tile([128, 512], F32, name="pl", tag="pl")
            nc.tensor.matmul(wps[:, 0:256], warm[:, 0:128], warm[:, 128:384],
                             start=True, stop=True)

        # probs tiles (tiny now): t0 first on scalar, rest follow there
        ptxs = []
        for t in range(NT):
            t_ = const.tile([128, PW], F16, name=f"ptx{t}", tag=f"ptx{t}")
            nc.scalar.dma_start(t_[:], ptx_d[t * 128:(t + 1) * 128, :])
            ptxs.append(t_)

        # sync queue: scatter matrix + remaining X pairs
        ss = const.tile([128, NCH * 128], F16, name="ss", tag="ss")
        nc.sync.dma_start(ss[:], s_d[:])
        xtile = [None] * NT     # (tile, col_base) per t
        xtile[0] = (wg, 0)
        xp0 = const.tile([128, 4096], F16, name="xp0", tag="xp0")
        nc.sync.dma_start(xp0[:], xk_d[0:128, :])
        xtile[1] = (xp0, 0)
        xtile[2] = (xp0, 2048)
        for bi in range(1, 3):
            t_ = const.tile([128, 4096], F16, name=f"xp{bi}", tag=f"xp{bi}")
            nc.sync.dma_start(t_[:], xk_d[bi * 128:(bi + 1) * 128, :])
            xtile[2 * bi + 1] = (t_, 0)
            xtile[2 * bi + 2] = (t_, 2048)
        x7 = const.tile([128, 2048], F16, name="x7", tag="x7")
        nc.sync.dma_start(x7[:], xk_d[3 * 128:4 * 128, 0:2048])
        xtile[7] = (x7, 0)

        def w_ap(j, k):
            if j == 0:
                return wg[:, KF * 512 + k * 128:KF * 512 + (k + 1) * 128]
            return wr[j][:, k * 128:(k + 1) * 128]

        wtj_prev = [None] * NCH
        wtj_cur = [None] * NCH
        obt = None
        for t in range(NT + 1):
            if t >= 1:
                obt = sbO.tile([128, NCH * 512], F16, name="obt", tag="obt")
            for j in range(NCH):
                if t < NT:
                    xt_, xb = xtile[t]
                    pg = psG.tile([128, 512], F32, name="pg", tag="pg")
                    for k in range(KF):
                        nc.tensor.matmul(pg[:], w_ap(j, k),
                                         xt_[:, xb + k * 512:
                                             xb + (k + 1) * 512],
                                         start=(k == 0), stop=(k == KF - 1))
                    wtj = sbW.tile([128, 512], F16, name="wtj", tag="wtj")
                    nc.vector.scalar_tensor_tensor(
                        wtj[:], pg[:], ptxs[t][:, 512 + j:513 + j],
                        ptxs[t][:, 0:512],
                        op0=mybir.AluOpType.add, op1=mybir.AluOpType.mult)
                    wtj_cur[j] = wtj
                if t >= 1:
                    pl = psL.tile([128, 512], F32, name="pl", tag="pl")
                    nc.tensor.matmul(pl[:], ss[:, j * 128:(j + 1) * 128],
                                     wtj_prev[j][:], start=True, stop=True)
                    # tail iteration: split drains across ACT and DVE
                    if t == NT and j % 2 == 1:
                        nc.vector.tensor_copy(obt[:, j * 512:(j + 1) * 512],
                                              pl[:])
                    else:
                        nc.scalar.activation(obt[:, j * 512:(j + 1) * 512],
                                             pl[:],
                                             mybir.ActivationFunctionType.Copy,
                                             bias=0.0, scale=1.0)
                    if t == NT and j % 2 == 1:
                        # final tile: quarter DMAs as strips complete
                        col = (t - 1) * NCH * 512 + (j - 1) * 512
                        oeng = nc.sync if j % 4 == 1 else nc.scalar
                        oeng.dma_start(out_d[:, col:col + 1024],
                                       obt[:, (j - 1) * 512:(j + 1) * 512])
            if t >= 1 and t < NT:
                col = (t - 1) * NCH * 512
                oeng = nc.sync if t % 2 == 0 else nc.scalar
                oeng.dma_start(out_d[:, col:col + NCH * 512], obt[:])
            if t < NT:
                wtj_prev, wtj_cur = wtj_cur, wtj_prev
    nc.finalize()
    return nc


def kernel(features, group_probs, W, b, label_ids):
    global LAST_EXEC_NS
    features = np.asarray(features, dtype=np.float32)
    group_probs = np.asarray(group_probs, dtype=np.float32)
    prep = _host_prep(W, b, label_ids)
    ws, bins = prep["ws"], prep["bins"]
    nc = _build_program()

    XT = features.T.astype(np.float16)                        # [F, B]
    PT = group_probs.T.astype(np.float16)                     # [G, B]
    WTf = prep["WT"]                                          # [F, 1024]
    # j-major W blocks, col j*512 + k*128 = WTf[k*128.., j*128..]
    wj = np.empty((128, NCH * KF * 128), dtype=np.float16)
    for j in range(NCH):
        for k in range(KF):
            wj[:, j * 512 + k * 128:j * 512 + (k + 1) * 128] = \
                WTf[k * 128:(k + 1) * 128, j * 128:(j + 1) * 128]
    bias16 = prep["biasT"].astype(np.float16)                 # [128, NCH]
    in_maps = []
    for c in range(NCORE):
        # k-interleaved X^T: xflat[t, p, k*512+cc] = XT[k*128+p, t*512+cc]
        xc = XT[:, c * BC:(c + 1) * BC].reshape(KF, 128, NT, 512)
        xflat = xc.transpose(2, 1, 0, 3).reshape(NT, 128, KF * 512)
        # wg = X t0 + W chunk 0; wrest = W chunks 1..7
        wgc = np.concatenate([xflat[0], wj[:, 0:512]], axis=1)
        wrestc = np.ascontiguousarray(wj[:, 512:])
        # xk row-blocks: (t1,t2), (t3,t4), (t5,t6), (t7, zero-pad)
        xk = np.zeros(((NT // 2) * 128, 2 * KF * 512), dtype=np.float16)
        for bi in range(3):
            xk[bi * 128:(bi + 1) * 128, :2048] = xflat[2 * bi + 1]
            xk[bi * 128:(bi + 1) * 128, 2048:] = xflat[2 * bi + 2]
        xk[3 * 128:4 * 128, :2048] = xflat[7]
        xk = np.ascontiguousarray(xk)
        # shared probs tile: row r -> group r//GPC, plus bias cols
        ptc = PT[:, c * BC:(c + 1) * BC].reshape(G, NT, 512)  # [16, 8, 512]
        ptx = np.empty((NT, 128, PW), dtype=np.float16)
        ptx[:, :, 0:512] = np.repeat(ptc, GPC, axis=0).transpose(1, 0, 2)
        ptx[:, :, 512:PW] = bias16[None, :, :]
        in_maps.append({
            "xk": xk,
            "ptx": np.ascontiguousarray(ptx.reshape(NT * 128, PW)),
            "wg": np.ascontiguousarray(wgc),
            "wrest": wrestc,
            "s": prep["SS"],
        })

    trace = bool(os.environ.get("BASS_TRACE"))
    if trace:
        bass_utils.upload_artifacts = lambda d: "local://skipped"
    try:
        res = bass_utils.run_bass_kernel_spmd(nc, in_maps,
                                              core_ids=list(range(NCORE)))
    except Exception:
        # transient NRT device errors have been observed; one retry
        res = bass_utils.run_bass_kernel_spmd(nc, in_maps,
                                              core_ids=list(range(NCORE)))
    if trace:
        LAST_EXEC_NS = res.exec_time_ns
        if res.exec_time_ns is not None:
            print(f"HW exec time: {res.exec_time_ns} ns")

    out = np.zeros((B, C), dtype=np.float32)
    for c in range(NCORE):
        o2 = res.results[c]["logits"]                          # [128, 32768]
        r0 = c * BC
        for t in range(NT):
            for j in range(NCH):
                col = (t * NCH + j) * 512
                strip = o2[0:ws[j], col:col + 512]             # [w_j, 512]
                out[r0 + t * 512:r0 + (t + 1) * 512, bins[j]] = \
                    strip.T.astype(np.float32)
    return out
